# revision 29
# baseline (speedup 1.0000x reference)
"""Trainium2 Bass kernel for a decoder block (MHA + GELU MLP, pre-LN, causal).

Problem shapes (hardcoded): B=2, T=2048, C=512, H=8, HD=64, f32 in/out.

Sharding: 8 cores = 2 batches x 4 query-blocks of 512 tokens. Every core
receives its batch's full x *rotated* so that its query block sits at token
rows 1536:2048 — the SPMD program is identical across cores while the causal
structure moves into per-core input data:
  - a per-slot exp bias (0 or -1e30) kills fully-masked 128-token key tiles,
  - a 0/1 mask multiplied into the probabilities (vector engine) handles the
    diagonal 512x512 block, identical for every core.

All matmuls run in bf16 (f32 PSUM accumulation). Engine balance:
  - PE: transposes, QKV, scores, PV, denominator broadcast, proj, FFN.
  - ACT (scalar): softmax exp on paired [128,2,512] two-bank PSUM tiles,
    LN normalize (per-partition scale+bias), PSUM transpose evictions, gelu.
  - DVE (vector): LN stats, K/V/Q evictions, diagonal mask multiplies,
    denominator reciprocal, residual adds.
DMA issue order feeds LN1 first (x tiles before weights, FFN weights last).
"""

import os
import sys

for _p in ("/opt/trn_rl_repo",):
    if _p not in sys.path and os.path.isdir(_p):
        sys.path.insert(0, _p)

import ml_dtypes
import numpy as np

import concourse.bacc as bacc
import concourse.bass as bass
import concourse.tile as tile
from concourse import mybir
from concourse.bass_utils import run_bass_kernel_spmd

F32 = mybir.dt.float32
BF16 = mybir.dt.bfloat16
AF = mybir.ActivationFunctionType

B, T, C, H, HD = 2, 2048, 512, 8, 64
NCORES = 8
QB = 512          # query-block tokens per core
NT = T // 128     # 16 key tiles
NQ = QB // 128    # 4 query tiles per core
NEG = -1.0e30

last_run = None       # test harness reads exec_time_ns from here
_prog_cache = {}


def _build_program(with_qkv_bias):
    nc = bacc.Bacc("TRN2", target_bir_lowering=False, debug=False,
                   num_devices=NCORES)

    xb_d = nc.dram_tensor("xb", [T, C], F32, kind="ExternalInput")
    wq_d = nc.dram_tensor("wq", [128, 4, 512], BF16, kind="ExternalInput")
    wk_d = nc.dram_tensor("wk", [128, 4, 512], BF16, kind="ExternalInput")
    wv_d = nc.dram_tensor("wv", [128, 4, 512], BF16, kind="ExternalInput")
    wo_d = nc.dram_tensor("wo", [128, 4, 512], BF16, kind="ExternalInput")
    w1_d = nc.dram_tensor("w1", [128, 16, 512], BF16, kind="ExternalInput")
    w2_d = nc.dram_tensor("w2", [128, 16, 512], BF16, kind="ExternalInput")
    kb_d = nc.dram_tensor("kbias", [128, 16], F32, kind="ExternalInput")
    bo_d = nc.dram_tensor("bo", [1, 512], BF16, kind="ExternalInput")
    b1_d = nc.dram_tensor("b1c", [128, 16], F32, kind="ExternalInput")
    b2_d = nc.dram_tensor("b2r", [1, 512], BF16, kind="ExternalInput")
    id_d = nc.dram_tensor("identc", [128, 128], BF16, kind="ExternalInput")
    mk_d = nc.dram_tensor("maskc", [128, 4, 2, 512], BF16, kind="ExternalInput")
    on_d = nc.dram_tensor("onesc", [128, 512], BF16, kind="ExternalInput")
    bq_d = (nc.dram_tensor("bqkv", [3, 1, 512], BF16, kind="ExternalInput")
            if with_qkv_bias else None)
    out_d = nc.dram_tensor("out", [QB, C], F32, kind="ExternalOutput")

    with tile.TileContext(nc) as tc:
        with (
            tc.tile_pool(name="const", bufs=1) as const,
            tc.tile_pool(name="mid", bufs=1) as mid,
            tc.tile_pool(name="tp", bufs=3) as tp,
            tc.tile_pool(name="sp", bufs=4) as sp,
        ):
            # ---------------- DMAs, ordered by first use ----------------
            ident = const.tile([128, 128], BF16)
            nc.sync.dma_start(ident[:], id_d[:])

            x_sb = mid.tile([128, 16, 512], F32)     # raw x (rotated)
            for t in range(12, 16):
                nc.sync.dma_start(x_sb[:, t, :], xb_d[bass.ts(t, 128), :])
            wq_sb = const.tile([128, 4, 512], BF16)
            nc.sync.dma_start(wq_sb[:], wq_d[:])
            for t in range(0, 4):
                nc.sync.dma_start(x_sb[:, t, :], xb_d[bass.ts(t, 128), :])
            wk_sb = const.tile([128, 4, 512], BF16)
            nc.sync.dma_start(wk_sb[:], wk_d[:])
            for t in range(4, 8):
                nc.sync.dma_start(x_sb[:, t, :], xb_d[bass.ts(t, 128), :])
            wv_sb = const.tile([128, 4, 512], BF16)
            nc.sync.dma_start(wv_sb[:], wv_d[:])
            for t in range(8, 12):
                nc.sync.dma_start(x_sb[:, t, :], xb_d[bass.ts(t, 128), :])

            kb_sb = const.tile([128, 16], F32)
            nc.sync.dma_start(kb_sb[:], kb_d[:])
            ones512 = const.tile([128, 512], BF16)
            nc.sync.dma_start(ones512[:], on_d[:])
            ones_sb = ones512  # [1, ...] slices come from row 0
            # 0/1 causal mask for the diagonal block (both halves duplicated)
            mask_sb = const.tile([128, 4, 2, 512], BF16)
            nc.sync.dma_start(mask_sb[:], mk_d[:])
            wo_sb = const.tile([128, 4, 512], BF16)
            nc.sync.dma_start(wo_sb[:], wo_d[:])
            bo_sb = const.tile([1, 512], BF16)
            nc.sync.dma_start(bo_sb[:], bo_d[:])
            b1_sb = const.tile([128, 16], F32)
            nc.sync.dma_start(b1_sb[:], b1_d[:])
            b2_sb = const.tile([1, 512], BF16)
            nc.sync.dma_start(b2_sb[:], b2_d[:])
            if with_qkv_bias:
                bq_sb = const.tile([3, 1, 512], BF16)
                nc.sync.dma_start(bq_sb[:], bq_d[:])
            w1_sb = const.tile([128, 16, 512], BF16)
            nc.sync.dma_start(w1_sb[:], w1_d[:])
            w2_sb = const.tile([128, 16, 512], BF16)
            nc.sync.dma_start(w2_sb[:], w2_d[:])

            eps_sb = const.tile([128, 1], F32)
            nc.vector.memset(eps_sb[:], 1e-5)

            # ---------------- persistent mid tensors ----------------
            h1t_sb = mid.tile([128, 4, 2048], BF16)  # ln1(x)^T
            kt_sb = mid.tile([128, 4, 2048], BF16)   # K^T  (head pair, 64h+d)
            v_sb = mid.tile([128, 16, 520], BF16)    # V + ones column per head
            qt_sb = mid.tile([128, 4, 512], BF16)    # Q^T
            at_sb = mid.tile([128, 4, 512], BF16)    # attnT (scaled)
            x2_sb = mid.tile([128, 4, 512], F32)     # post-attn residual
            h2t_sb = mid.tile([128, 4, 512], BF16)   # ln2(x2)^T

            # pre-set the ones columns of V (col 64 of each 65-wide group)
            vones = (v_sb[:, :, :]
                     .rearrange("p a (h e) -> p a h e", e=65)[:, :, :, 64:65])
            nc.vector.tensor_copy(
                vones, ones512[:, 0:128]
                .rearrange("p (a h) -> p a h", h=8).unsqueeze(3))

            def layernorm_to(src_ap, dst_ap, scalar_norm=True):
                # stats on vector; normalize on scalar (per-partition
                # scale+bias ht = x*rs + (-mu*rs)) or on vector when the
                # scalar queue must stay short (LN1 gates the first exp)
                st = sp.tile([128, 6], F32, tag="st")
                nc.vector.bn_stats(out=st[:], in_=src_ap)
                mv = sp.tile([128, 2], F32, tag="mv")
                nc.vector.bn_aggr(out=mv[:], in_=st[:])
                lg = sp.tile([128, 1], F32, tag="lg")
                nc.scalar.activation(out=lg[:], in_=mv[:, 1:2], func=AF.Sqrt,
                                     bias=eps_sb[:])
                rs = sp.tile([128, 1], F32, tag="rs")
                nc.vector.reciprocal(out=rs[:], in_=lg[:])
                if scalar_norm:
                    nmr = sp.tile([128, 1], F32, tag="nmr")
                    nc.vector.tensor_scalar(
                        out=nmr[:], in0=mv[:, 0:1], scalar1=rs[:],
                        scalar2=-1.0, op0=mybir.AluOpType.mult,
                        op1=mybir.AluOpType.mult)
                    nc.scalar.activation(out=dst_ap, in_=src_ap,
                                         func=AF.Identity,
                                         bias=nmr[:], scale=rs[:])
                else:
                    nc.vector.tensor_scalar(
                        out=dst_ap, in0=src_ap, scalar1=mv[:, 0:1],
                        scalar2=rs[:], op0=mybir.AluOpType.subtract,
                        op1=mybir.AluOpType.mult)

            # ==== LN1 + Q/K/V + attention, block-interleaved ====
            # One shared 2-deep ring of [128,2,512] f32 PSUM tiles (4 banks)
            # carries Q/K/V production pairs AND score tiles; the PV
            # accumulators take the other 4 banks (transposes borrow 2 banks
            # only during the lead-in). K^T/V production for chunks 2-3 is
            # issued as dense 16-MM blocks at the s=8 / s=12 boundaries of
            # attention pass 1 (prs 0,1), so softmax exp starts ~25us in
            # instead of after all production; pass 2 (prs 2,3) pre-issues
            # its first two score stages before pass 1's denominator tails
            # so the scalar exp stream never drains.
            with (
                tc.tile_pool(name="ps2", bufs=2, space="PSUM") as ps2_ps,
                tc.tile_pool(name="ap", bufs=6) as ap_pool,
            ):
                def ring():
                    return ps2_ps.tile([128, 2, 512], F32, tag="ps",
                                       name="ring")

                ptr1_box = [None]

                def ln1_tile(t):
                    ht = tp.tile([128, 512], BF16, tag="ht")
                    layernorm_to(x_sb[:, t, :], ht[:], scalar_norm=False)
                    pst = ptr1_box[0].tile([128, 4, 128], BF16, tag="tr",
                                           name="pst")
                    for cc in range(4):
                        nc.tensor.transpose(
                            pst[:, cc, :], ht[:, bass.ts(cc, 128)], ident[:])
                    ev = h1t_sb[:, :, bass.ts(t, 128)]
                    nc.scalar.copy(ev, pst[:])

                def qt_pair(i):
                    rt = ring()
                    for j in range(2):
                        pr = 2 * i + j
                        for cc in range(4):
                            nc.tensor.matmul(
                                rt[:, j, :], wq_sb[:, cc, bass.ts(pr, 128)],
                                h1t_sb[:, cc, 1536:2048],
                                start=(cc == 0),
                                stop=(cc == 3 and not with_qkv_bias))
                        if with_qkv_bias:
                            nc.tensor.matmul(
                                rt[:, j, :], bq_sb[0, :, bass.ts(pr, 128)],
                                ones512[:], start=False, stop=True)
                    nc.vector.tensor_copy(qt_sb[:, 2 * i:2 * i + 2, :], rt[:])

                def kt_pair(nk, i):
                    rt = ring()
                    for j in range(2):
                        pr = 2 * i + j
                        for cc in range(4):
                            nc.tensor.matmul(
                                rt[:, j, :], wk_sb[:, cc, bass.ts(pr, 128)],
                                h1t_sb[:, cc, bass.ts(nk, 512)],
                                start=(cc == 0),
                                stop=(cc == 3 and not with_qkv_bias))
                        if with_qkv_bias:
                            nc.tensor.matmul(
                                rt[:, j, :], bq_sb[1, :, bass.ts(pr, 128)],
                                ones512[:], start=False, stop=True)
                    ev = kt_sb[:, 2 * i:2 * i + 2, bass.ts(nk, 512)]
                    nc.vector.tensor_copy(ev, rt[:])

                def v_pair(t):
                    rt = ring()
                    for j in range(2):
                        for cc in range(4):
                            nc.tensor.matmul(
                                rt[:, j, :],
                                h1t_sb[:, cc, bass.ts(t + j, 128)],
                                wv_sb[:, cc, :],
                                start=(cc == 0),
                                stop=(cc == 3 and not with_qkv_bias))
                        if with_qkv_bias:
                            nc.tensor.matmul(
                                rt[:, j, :], ones_sb[0:1, 0:128], bq_sb[2],
                                start=False, stop=True)
                    ev = (v_sb[:, t:t + 2, :]
                          .rearrange("p a (h e) -> p a h e", e=65)[:, :, :, 0:64])
                    sv = rt[:].rearrange("p a (h e) -> p a h e", e=64)
                    nc.vector.tensor_copy(ev, sv)

                # Attention helpers; PV accumulators (po) are allocated
                # only once the lead-in transpose pool has returned its banks.
                LAG = 2
                po = {}
                pts = {}

                def pv_pair(pr, s):
                    pt_s = pts.pop((pr, s))
                    for half in range(2):
                        h = 2 * pr + half
                        nc.tensor.matmul(
                            po[pr][half][:],
                            v_sb[:, s, h * 65:(h + 1) * 65],
                            pt_s[:, half, :],
                            start=(s == 0), stop=(s == NT - 1),
                            skip_group_check=True)

                def score_exp(pr, s):
                    pss = ring()
                    for half in range(2):
                        base = 64 * half
                        nc.tensor.matmul(
                            pss[:, half, :],
                            kt_sb[base:base + 64, pr, bass.ts(s, 128)],
                            qt_sb[base:base + 64, pr, :],
                            start=True, stop=True)
                    ptile = ap_pool.tile([128, 2, 512], BF16, tag="pt",
                                         bufs=10)
                    nc.scalar.activation(
                        out=ptile[:], in_=pss[:], func=AF.Exp,
                        bias=kb_sb[:, s:s + 1])
                    if s >= 12:
                        mt = ap_pool.tile([128, 2, 512], BF16, tag="mt",
                                          bufs=4)
                        nc.vector.tensor_mul(
                            out=mt[:], in0=ptile[:],
                            in1=mask_sb[:, s - 12, :, :])
                        ptile = mt
                    pts[(pr, s)] = ptile

                def attn_tail(pr):
                    for half in range(2):
                        base = 64 * half
                        dn = ap_pool.tile([1, 512], BF16, tag="dn",
                                          bufs=4)
                        with nc.allow_low_precision(
                                reason="softmax denominator to bf16; "
                                "~0.4% relative, inside the 2e-2 gate"):
                            nc.vector.tensor_copy(
                                dn[:], po[pr][half][64:65, :])
                        bc = ring()
                        nc.tensor.matmul(
                            bc[0:64, 0, :], ones_sb[0:1, 0:64], dn[:],
                            start=True, stop=True)
                        rb = ap_pool.tile([64, 512], F32, tag="rb",
                                          bufs=4)
                        nc.vector.reciprocal_approx_fast(
                            out=rb[:], in_=bc[0:64, 0, :])
                        nc.vector.tensor_mul(
                            out=at_sb[base:base + 64, pr, :],
                            in0=po[pr][half][0:64, :], in1=rb[:])

                # ---- lead-in + early attention (scores/exp only) ----
                # LN1 of tiles 8..11 and V tiles 8..9 ride between the first
                # attention stages instead of ahead of them, so the first
                # score matmul no longer queues behind their transposes.
                with tc.tile_pool(name="ptr1", bufs=2, space="PSUM") as ptr1:
                    ptr1_box[0] = ptr1
                    for t in range(12, 16):
                        ln1_tile(t)
                    qt_pair(0)
                    qt_pair(1)
                    for t in range(0, 4):
                        ln1_tile(t)
                    kt_pair(0, 0)
                    kt_pair(0, 1)
                    v_pair(0)
                    v_pair(2)
                    for t in range(4, 8):
                        ln1_tile(t)
                    kt_pair(1, 0)
                    kt_pair(1, 1)
                    v_pair(4)
                    v_pair(6)
                    EARLY = {0: 8, 1: 9, 2: 10, 3: 11}
                    for s in range(4):
                        score_exp(0, s)
                        score_exp(1, s)
                        if s in EARLY:
                            ln1_tile(EARLY[s])

                with tc.tile_pool(name="psO", bufs=2, space="PSUM") as po_ps:
                    def alloc_po(pr):
                        po[pr] = [po_ps.tile([65, 512], F32, tag=f"po{i}",
                                             name=f"po{pr}_{i}")
                                  for i in range(2)]

                    def attn_block(prs, s_lo, s_hi):
                        for s in range(s_lo, s_hi):
                            for pr in prs:
                                score_exp(pr, s)
                            if s >= LAG:
                                for pr in prs:
                                    pv_pair(pr, s - LAG)

                    def attn_drain(prs):
                        for s in range(NT - LAG, NT):
                            for pr in prs:
                                pv_pair(pr, s)

                    alloc_po(0)
                    alloc_po(1)
                    pv_next = 0
                    for s in range(4, NT):
                        score_exp(0, s)
                        score_exp(1, s)
                        if s == 4:
                            v_pair(8)
                        elif s == 6:
                            v_pair(10)
                        elif s == 7:
                            kt_pair(2, 0)
                            kt_pair(2, 1)
                        elif s == 9:
                            v_pair(12)
                        elif s == 10:
                            v_pair(14)
                        elif s == 11:
                            kt_pair(3, 0)
                            kt_pair(3, 1)
                        for _ in range(2):
                            if pv_next <= s - LAG:
                                pv_pair(0, pv_next)
                                pv_pair(1, pv_next)
                                pv_next += 1
                    while pv_next < NT:
                        pv_pair(0, pv_next)
                        pv_pair(1, pv_next)
                        pv_next += 1
                    # keep the exp stream fed through pass 1's tails
                    attn_block((2, 3), 0, LAG)
                    attn_tail(0)
                    attn_tail(1)
                    alloc_po(2)
                    alloc_po(3)
                    attn_block((2, 3), LAG, 16)
                    attn_drain((2, 3))
                    attn_tail(2)
                    attn_tail(3)

            # ======== output projection + residual + LN2 (one scope) ====
            with (
                tc.tile_pool(name="pf", bufs=2, space="PSUM") as pf_ps,
                tc.tile_pool(name="ptr2", bufs=2, space="PSUM") as ptr2,
            ):
                for qt in range(NQ):
                    ps = pf_ps.tile([128, 512], F32, tag="pf")
                    nc.tensor.matmul(ps[:], ones_sb[0:1, 0:128],
                                     bo_sb[:], start=True, stop=False)
                    for cc in range(4):
                        nc.tensor.matmul(
                            ps[:], at_sb[:, cc, bass.ts(qt, 128)],
                            wo_sb[:, cc, :], start=False, stop=(cc == 3))
                    nc.vector.tensor_add(out=x2_sb[:, qt, :], in0=ps[:],
                                         in1=x_sb[:, 12 + qt, :])
                    ht = tp.tile([128, 512], BF16, tag="ht")
                    layernorm_to(x2_sb[:, qt, :], ht[:])
                    pst = ptr2.tile([128, 4, 128], BF16, tag="tr")
                    for cc in range(4):
                        nc.tensor.transpose(
                            pst[:, cc, :], ht[:, bass.ts(cc, 128)], ident[:])
                    ev = h2t_sb[:, :, bass.ts(qt, 128)]
                    nc.scalar.copy(ev, pst[:])

            # ======== FFN1 + gelu + FFN2, fused per f-tile ========
            with (
                tc.tile_pool(name="pg", bufs=2, space="PSUM") as pg_ps,
                tc.tile_pool(name="pf2", bufs=1, space="PSUM") as pf2_ps,
                tc.tile_pool(name="gp", bufs=3) as gp,
                tc.tile_pool(name="op", bufs=2) as op,
            ):
                pso = [pf2_ps.tile([128, 512], F32, tag=f"o{qt}",
                                   name=f"o{qt}") for qt in range(NQ)]
                for qt in range(NQ):
                    nc.tensor.matmul(
                        pso[qt][:], ones_sb[0:1, 0:128], b2_sb[:],
                        start=True, stop=False, skip_group_check=True)
                for f in range(16):
                    ps = pg_ps.tile([128, 512], F32, tag="pg")
                    for cc in range(4):
                        nc.tensor.matmul(
                            ps[:], w1_sb[:, f, bass.ts(cc, 128)],
                            h2t_sb[:, cc, :],
                            start=(cc == 0), stop=(cc == 3))
                    gt = gp.tile([128, 512], BF16, tag="gt")
                    nc.scalar.activation(
                        out=gt[:], in_=ps[:], func=AF.Gelu,
                        bias=b1_sb[:, f:f + 1])
                    for qt in range(NQ):
                        nc.tensor.matmul(
                            pso[qt][:],
                            gt[:, bass.ts(qt, 128)],
                            w2_sb[:, f, :], start=False, stop=(f == 15),
                            skip_group_check=True)
                for qt in range(NQ):
                    ot = op.tile([128, 512], F32, tag="ot")
                    nc.vector.tensor_add(out=ot[:], in0=pso[qt][:],
                                         in1=x2_sb[:, qt, :])
                    eng = nc.sync if qt % 2 == 0 else nc.scalar
                    eng.dma_start(out_d[bass.ts(qt, 128), :], ot[:])

    nc.compile()
    return nc


def _bf16(a):
    return np.ascontiguousarray(np.asarray(a, np.float32)).astype(
        ml_dtypes.bfloat16)


def _host_prep(x, Wq, Wk, Wv, Wo, bo, W1, b1, W2, b2, g1, be1, g2, be2):
    """Fold LN gains into weights; build per-core rotated inputs/slot biases."""
    x = np.asarray(x, np.float32)
    g1 = np.asarray(g1, np.float32)
    be1 = np.asarray(be1, np.float32)
    g2 = np.asarray(g2, np.float32)
    be2 = np.asarray(be2, np.float32)

    wq_cat = np.transpose(np.asarray(Wq, np.float32), (1, 0, 2)).reshape(C, H * HD)
    wk_cat = np.transpose(np.asarray(Wk, np.float32), (1, 0, 2)).reshape(C, H * HD)
    wv_cat = np.transpose(np.asarray(Wv, np.float32), (1, 0, 2)).reshape(C, H * HD)
    scl = float(HD) ** -0.5
    wq_f = (g1[:, None] * wq_cat) * scl
    wk_f = g1[:, None] * wk_cat
    wv_f = g1[:, None] * wv_cat
    bq = (be1 @ wq_cat) * scl
    bk = be1 @ wk_cat
    bv = be1 @ wv_cat
    with_qkv_bias = bool(np.any(bq) or np.any(bk) or np.any(bv))

    W1 = np.asarray(W1, np.float32)
    w1_f = g2[:, None] * W1
    b1_f = np.asarray(b1, np.float32) + be2 @ W1

    # 0/1 keep-mask for the diagonal block in S^T layout, both halves
    maskc = np.zeros((4, 128, 512), np.float32)
    qidx = np.arange(512)[None, :]
    for dd in range(4):
        pidx = 128 * dd + np.arange(128)[:, None]
        maskc[dd] = (pidx <= qidx).astype(np.float32)
    mask4 = np.broadcast_to(
        maskc.transpose(1, 0, 2)[:, :, None, :], (128, 4, 2, 512))

    common = {
        "identc": _bf16(np.eye(128, dtype=np.float32)),
        "maskc": _bf16(mask4),
        "onesc": _bf16(np.ones((128, 512), np.float32)),
        "wq": _bf16(wq_f.reshape(4, 128, 512).transpose(1, 0, 2)),
        "wk": _bf16(wk_f.reshape(4, 128, 512).transpose(1, 0, 2)),
        "wv": _bf16(wv_f.reshape(4, 128, 512).transpose(1, 0, 2)),
        "wo": _bf16(
            np.asarray(Wo, np.float32).reshape(4, 128, 512).transpose(1, 0, 2)),
        "w1": _bf16(
            np.transpose(
                w1_f.reshape(4, 128, 16, 128).transpose(2, 1, 0, 3)
                .reshape(16, 128, 512), (1, 0, 2))),
        "w2": _bf16(np.transpose(
            np.asarray(W2, np.float32).reshape(16, 128, 512), (1, 0, 2))),
        "bo": _bf16(np.asarray(bo, np.float32).reshape(1, 512)),
        "b1c": np.ascontiguousarray(b1_f.reshape(16, 128).T),
        "b2r": _bf16(np.asarray(b2, np.float32).reshape(1, 512)),
    }
    if with_qkv_bias:
        common["bqkv"] = _bf16(np.stack([bq, bk, bv]).reshape(3, 1, 512))

    in_maps = []
    for c in range(NCORES):
        bb, j = c // 4, c % 4
        o = QB * j
        xb_rot = np.roll(x[bb], 1536 - o, axis=0)
        kbias = np.zeros(16, np.float32)
        for s in range(12):
            m = (s - 12 + 4 * j) % 16
            if m >= 4 * j:       # original key tile at/after the q block
                kbias[s] = NEG
        im = dict(common)
        im["xb"] = np.ascontiguousarray(xb_rot)
        im["kbias"] = np.ascontiguousarray(
            np.broadcast_to(kbias.reshape(1, 16), (128, 16)))
        in_maps.append(im)
    return in_maps, with_qkv_bias


def kernel(**inputs):
    global last_run
    in_maps, with_qkv_bias = _host_prep(**inputs)
    if with_qkv_bias not in _prog_cache:
        _prog_cache[with_qkv_bias] = _build_program(with_qkv_bias)
    nc = _prog_cache[with_qkv_bias]
    res = run_bass_kernel_spmd(nc, in_maps, list(range(NCORES)))
    last_run = res
    out = np.empty((B, T, C), np.float32)
    for c in range(NCORES):
        bb, j = c // 4, c % 4
        out[bb, QB * j:QB * (j + 1), :] = res.results[c]["out"]
    return out


# revision 30
# speedup vs baseline: 1.0238x; 1.0238x over previous
"""Trainium2 Bass kernel for a decoder block (MHA + GELU MLP, pre-LN, causal).

Problem shapes (hardcoded): B=2, T=2048, C=512, H=8, HD=64, f32 in/out.

Sharding: 8 cores = 2 batches x 4 query-blocks of 512 tokens. Every core
receives its batch's full x *rotated* so that its query block sits at token
rows 1536:2048 — the SPMD program is identical across cores while the causal
structure moves into per-core input data:
  - a per-slot exp bias (0 or -1e30) kills fully-masked 128-token key tiles,
  - a 0/1 mask multiplied into the probabilities (vector engine) handles the
    diagonal 512x512 block, identical for every core.

All matmuls run in bf16 (f32 PSUM accumulation). Engine balance:
  - PE: transposes, QKV, scores, PV, denominator broadcast, proj, FFN.
  - ACT (scalar): softmax exp on paired [128,2,512] two-bank PSUM tiles,
    LN normalize (per-partition scale+bias), PSUM transpose evictions, gelu.
  - DVE (vector): LN stats, K/V/Q evictions, diagonal mask multiplies,
    denominator reciprocal, residual adds.
DMA issue order feeds LN1 first (x tiles before weights, FFN weights last).
"""

import os
import sys

for _p in ("/opt/trn_rl_repo",):
    if _p not in sys.path and os.path.isdir(_p):
        sys.path.insert(0, _p)

import ml_dtypes
import numpy as np

import concourse.bacc as bacc
import concourse.bass as bass
import concourse.tile as tile
from concourse import mybir
from concourse.bass_utils import run_bass_kernel_spmd

F32 = mybir.dt.float32
BF16 = mybir.dt.bfloat16
AF = mybir.ActivationFunctionType

B, T, C, H, HD = 2, 2048, 512, 8, 64
NCORES = 8
QB = 512          # query-block tokens per core
NT = T // 128     # 16 key tiles
NQ = QB // 128    # 4 query tiles per core
NEG = -1.0e30

last_run = None       # test harness reads exec_time_ns from here
_prog_cache = {}


def _build_program(with_qkv_bias):
    nc = bacc.Bacc("TRN2", target_bir_lowering=False, debug=False,
                   num_devices=NCORES)

    xb_d = nc.dram_tensor("xb", [T, C], F32, kind="ExternalInput")
    wq_d = nc.dram_tensor("wq", [128, 4, 512], BF16, kind="ExternalInput")
    wk_d = nc.dram_tensor("wk", [128, 4, 512], BF16, kind="ExternalInput")
    wv_d = nc.dram_tensor("wv", [128, 4, 512], BF16, kind="ExternalInput")
    wo_d = nc.dram_tensor("wo", [128, 4, 512], BF16, kind="ExternalInput")
    w1_d = nc.dram_tensor("w1", [128, 16, 512], BF16, kind="ExternalInput")
    w2_d = nc.dram_tensor("w2", [128, 16, 512], BF16, kind="ExternalInput")
    kb_d = nc.dram_tensor("kbias", [128, 16], F32, kind="ExternalInput")
    bo_d = nc.dram_tensor("bo", [1, 512], BF16, kind="ExternalInput")
    b1_d = nc.dram_tensor("b1c", [128, 16], F32, kind="ExternalInput")
    b2_d = nc.dram_tensor("b2r", [1, 512], BF16, kind="ExternalInput")
    id_d = nc.dram_tensor("identc", [128, 128], BF16, kind="ExternalInput")
    mk_d = nc.dram_tensor("maskc", [128, 4, 2, 512], BF16, kind="ExternalInput")
    on_d = nc.dram_tensor("onesc", [128, 512], BF16, kind="ExternalInput")
    bq_d = (nc.dram_tensor("bqkv", [3, 1, 512], BF16, kind="ExternalInput")
            if with_qkv_bias else None)
    out_d = nc.dram_tensor("out", [QB, C], F32, kind="ExternalOutput")

    with tile.TileContext(nc) as tc:
        with (
            tc.tile_pool(name="const", bufs=1) as const,
            tc.tile_pool(name="mid", bufs=1) as mid,
            tc.tile_pool(name="tp", bufs=3) as tp,
            tc.tile_pool(name="sp", bufs=4) as sp,
        ):
            # ---------------- DMAs, ordered by first use ----------------
            ident = const.tile([128, 128], BF16)
            nc.sync.dma_start(ident[:], id_d[:])

            x_sb = mid.tile([128, 16, 512], F32)     # raw x (rotated)
            for t in range(12, 16):
                nc.sync.dma_start(x_sb[:, t, :], xb_d[bass.ts(t, 128), :])
            wq_sb = const.tile([128, 4, 512], BF16)
            nc.sync.dma_start(wq_sb[:], wq_d[:])
            for t in range(0, 4):
                nc.sync.dma_start(x_sb[:, t, :], xb_d[bass.ts(t, 128), :])
            wk_sb = const.tile([128, 4, 512], BF16)
            nc.sync.dma_start(wk_sb[:], wk_d[:])
            for t in range(4, 8):
                nc.sync.dma_start(x_sb[:, t, :], xb_d[bass.ts(t, 128), :])
            wv_sb = const.tile([128, 4, 512], BF16)
            nc.sync.dma_start(wv_sb[:], wv_d[:])
            for t in range(8, 12):
                nc.sync.dma_start(x_sb[:, t, :], xb_d[bass.ts(t, 128), :])

            kb_sb = const.tile([128, 16], F32)
            nc.sync.dma_start(kb_sb[:], kb_d[:])
            ones512 = const.tile([128, 512], BF16)
            nc.sync.dma_start(ones512[:], on_d[:])
            ones_sb = ones512  # [1, ...] slices come from row 0
            # 0/1 causal mask for the diagonal block (both halves duplicated)
            mask_sb = const.tile([128, 4, 2, 512], BF16)
            nc.sync.dma_start(mask_sb[:], mk_d[:])
            wo_sb = const.tile([128, 4, 512], BF16)
            nc.sync.dma_start(wo_sb[:], wo_d[:])
            bo_sb = const.tile([1, 512], BF16)
            nc.sync.dma_start(bo_sb[:], bo_d[:])
            b1_sb = const.tile([128, 16], F32)
            nc.sync.dma_start(b1_sb[:], b1_d[:])
            b2_sb = const.tile([1, 512], BF16)
            nc.sync.dma_start(b2_sb[:], b2_d[:])
            if with_qkv_bias:
                bq_sb = const.tile([3, 1, 512], BF16)
                nc.sync.dma_start(bq_sb[:], bq_d[:])
            w1_sb = const.tile([128, 16, 512], BF16)
            nc.sync.dma_start(w1_sb[:], w1_d[:])
            w2_sb = const.tile([128, 16, 512], BF16)
            nc.sync.dma_start(w2_sb[:], w2_d[:])

            eps_sb = const.tile([128, 1], F32)
            nc.vector.memset(eps_sb[:], 1e-5)

            # ---------------- persistent mid tensors ----------------
            h1t_sb = mid.tile([128, 4, 2048], BF16)  # ln1(x)^T
            kt_sb = mid.tile([128, 4, 2048], BF16)   # K^T  (head pair, 64h+d)
            v_sb = mid.tile([128, 16, 520], BF16)    # V + ones column per head
            qt_sb = mid.tile([128, 4, 512], BF16)    # Q^T
            at_sb = mid.tile([128, 4, 512], BF16)    # attnT (scaled)
            x2_sb = mid.tile([128, 4, 512], F32)     # post-attn residual
            h2t_sb = mid.tile([128, 4, 512], BF16)   # ln2(x2)^T

            # pre-set the ones columns of V (col 64 of each 65-wide group)
            vones = (v_sb[:, :, :]
                     .rearrange("p a (h e) -> p a h e", e=65)[:, :, :, 64:65])
            nc.vector.tensor_copy(
                vones, ones512[:, 0:128]
                .rearrange("p (a h) -> p a h", h=8).unsqueeze(3))

            def layernorm_to(src_ap, dst_ap, scalar_norm=True):
                # stats on vector; normalize on scalar (per-partition
                # scale+bias ht = x*rs + (-mu*rs)) or on vector when the
                # scalar queue must stay short (LN1 gates the first exp)
                st = sp.tile([128, 6], F32, tag="st")
                nc.vector.bn_stats(out=st[:], in_=src_ap)
                mv = sp.tile([128, 2], F32, tag="mv")
                nc.vector.bn_aggr(out=mv[:], in_=st[:])
                lg = sp.tile([128, 1], F32, tag="lg")
                nc.scalar.activation(out=lg[:], in_=mv[:, 1:2], func=AF.Sqrt,
                                     bias=eps_sb[:])
                rs = sp.tile([128, 1], F32, tag="rs")
                nc.vector.reciprocal(out=rs[:], in_=lg[:])
                if scalar_norm:
                    nmr = sp.tile([128, 1], F32, tag="nmr")
                    nc.vector.tensor_scalar(
                        out=nmr[:], in0=mv[:, 0:1], scalar1=rs[:],
                        scalar2=-1.0, op0=mybir.AluOpType.mult,
                        op1=mybir.AluOpType.mult)
                    nc.scalar.activation(out=dst_ap, in_=src_ap,
                                         func=AF.Identity,
                                         bias=nmr[:], scale=rs[:])
                else:
                    nc.vector.tensor_scalar(
                        out=dst_ap, in0=src_ap, scalar1=mv[:, 0:1],
                        scalar2=rs[:], op0=mybir.AluOpType.subtract,
                        op1=mybir.AluOpType.mult)

            # ==== LN1 + Q/K/V + attention, block-interleaved ====
            # One shared 2-deep ring of [128,2,512] f32 PSUM tiles (4 banks)
            # carries Q/K/V production pairs AND score tiles; the PV
            # accumulators take the other 4 banks (transposes borrow 2 banks
            # only during the lead-in). K^T/V production for chunks 2-3 is
            # issued as dense 16-MM blocks at the s=8 / s=12 boundaries of
            # attention pass 1 (prs 0,1), so softmax exp starts ~25us in
            # instead of after all production; pass 2 (prs 2,3) pre-issues
            # its first two score stages before pass 1's denominator tails
            # so the scalar exp stream never drains.
            with (
                tc.tile_pool(name="ps2", bufs=2, space="PSUM") as ps2_ps,
                tc.tile_pool(name="ap", bufs=6) as ap_pool,
            ):
                def ring():
                    return ps2_ps.tile([128, 2, 512], F32, tag="ps",
                                       name="ring")

                ptr1_box = [None]

                def ln1_tile(t):
                    ht = tp.tile([128, 512], BF16, tag="ht")
                    layernorm_to(x_sb[:, t, :], ht[:], scalar_norm=False)
                    pst = ptr1_box[0].tile([128, 4, 128], BF16, tag="tr",
                                           name="pst")
                    for cc in range(4):
                        nc.tensor.transpose(
                            pst[:, cc, :], ht[:, bass.ts(cc, 128)], ident[:])
                    ev = h1t_sb[:, :, bass.ts(t, 128)]
                    nc.scalar.copy(ev, pst[:])

                def qt_pair(i):
                    rt = ring()
                    for j in range(2):
                        pr = 2 * i + j
                        for cc in range(4):
                            nc.tensor.matmul(
                                rt[:, j, :], wq_sb[:, cc, bass.ts(pr, 128)],
                                h1t_sb[:, cc, 1536:2048],
                                start=(cc == 0),
                                stop=(cc == 3 and not with_qkv_bias))
                        if with_qkv_bias:
                            nc.tensor.matmul(
                                rt[:, j, :], bq_sb[0, :, bass.ts(pr, 128)],
                                ones512[:], start=False, stop=True)
                    nc.vector.tensor_copy(qt_sb[:, 2 * i:2 * i + 2, :], rt[:])

                def kt_pair(nk, i):
                    rt = ring()
                    for j in range(2):
                        pr = 2 * i + j
                        for cc in range(4):
                            nc.tensor.matmul(
                                rt[:, j, :], wk_sb[:, cc, bass.ts(pr, 128)],
                                h1t_sb[:, cc, bass.ts(nk, 512)],
                                start=(cc == 0),
                                stop=(cc == 3 and not with_qkv_bias))
                        if with_qkv_bias:
                            nc.tensor.matmul(
                                rt[:, j, :], bq_sb[1, :, bass.ts(pr, 128)],
                                ones512[:], start=False, stop=True)
                    ev = kt_sb[:, 2 * i:2 * i + 2, bass.ts(nk, 512)]
                    nc.vector.tensor_copy(ev, rt[:])

                def v_pair(t):
                    rt = ring()
                    for j in range(2):
                        for cc in range(4):
                            nc.tensor.matmul(
                                rt[:, j, :],
                                h1t_sb[:, cc, bass.ts(t + j, 128)],
                                wv_sb[:, cc, :],
                                start=(cc == 0),
                                stop=(cc == 3 and not with_qkv_bias))
                        if with_qkv_bias:
                            nc.tensor.matmul(
                                rt[:, j, :], ones_sb[0:1, 0:128], bq_sb[2],
                                start=False, stop=True)
                    ev = (v_sb[:, t:t + 2, :]
                          .rearrange("p a (h e) -> p a h e", e=65)[:, :, :, 0:64])
                    sv = rt[:].rearrange("p a (h e) -> p a h e", e=64)
                    nc.vector.tensor_copy(ev, sv)

                # ---- lead-in: LN1 + Q^T + K chunks 0,1 + V tiles 0..7 ----
                with tc.tile_pool(name="ptr1", bufs=2, space="PSUM") as ptr1:
                    ptr1_box[0] = ptr1
                    for t in range(12, 16):
                        ln1_tile(t)
                    qt_pair(0)
                    qt_pair(1)
                    for t in range(0, 4):
                        ln1_tile(t)
                    kt_pair(0, 0)
                    kt_pair(0, 1)
                    v_pair(0)
                    v_pair(2)
                    for t in range(4, 8):
                        ln1_tile(t)
                    kt_pair(1, 0)
                    kt_pair(1, 1)
                    v_pair(4)
                    v_pair(6)
                    for t in range(8, 12):
                        ln1_tile(t)

                with tc.tile_pool(name="psO", bufs=2, space="PSUM") as po_ps:
                    LAG = 2
                    po = {}
                    pts = {}

                    def alloc_po(pr):
                        po[pr] = [po_ps.tile([65, 512], F32, tag=f"po{i}",
                                             name=f"po{pr}_{i}")
                                  for i in range(2)]

                    def pv_pair(pr, s):
                        pt_s = pts.pop((pr, s))
                        for half in range(2):
                            h = 2 * pr + half
                            nc.tensor.matmul(
                                po[pr][half][:],
                                v_sb[:, s, h * 65:(h + 1) * 65],
                                pt_s[:, half, :],
                                start=(s == 0), stop=(s == NT - 1),
                                skip_group_check=True)

                    def score_exp(pr, s):
                        pss = ring()
                        for half in range(2):
                            base = 64 * half
                            nc.tensor.matmul(
                                pss[:, half, :],
                                kt_sb[base:base + 64, pr, bass.ts(s, 128)],
                                qt_sb[base:base + 64, pr, :],
                                start=True, stop=True)
                        ptile = ap_pool.tile([128, 2, 512], BF16, tag="pt")
                        nc.scalar.activation(
                            out=ptile[:], in_=pss[:], func=AF.Exp,
                            bias=kb_sb[:, s:s + 1])
                        if s >= 12:
                            mt = ap_pool.tile([128, 2, 512], BF16, tag="mt")
                            nc.vector.tensor_mul(
                                out=mt[:], in0=ptile[:],
                                in1=mask_sb[:, s - 12, :, :])
                            ptile = mt
                        pts[(pr, s)] = ptile

                    def attn_tail(pr):
                        for half in range(2):
                            base = 64 * half
                            dn = ap_pool.tile([1, 512], BF16, tag="dn")
                            with nc.allow_low_precision(
                                    reason="softmax denominator to bf16; "
                                    "~0.4% relative, inside the 2e-2 gate"):
                                nc.vector.tensor_copy(
                                    dn[:], po[pr][half][64:65, :])
                            bc = ring()
                            nc.tensor.matmul(
                                bc[0:64, 0, :], ones_sb[0:1, 0:64], dn[:],
                                start=True, stop=True)
                            rb = ap_pool.tile([64, 512], F32, tag="rb")
                            nc.vector.reciprocal_approx_fast(
                                out=rb[:], in_=bc[0:64, 0, :])
                            nc.vector.tensor_mul(
                                out=at_sb[base:base + 64, pr, :],
                                in0=po[pr][half][0:64, :], in1=rb[:])

                    def attn_block(prs, s_lo, s_hi):
                        for s in range(s_lo, s_hi):
                            for pr in prs:
                                score_exp(pr, s)
                            if s >= LAG:
                                for pr in prs:
                                    pv_pair(pr, s - LAG)

                    def attn_drain(prs):
                        for s in range(NT - LAG, NT):
                            for pr in prs:
                                pv_pair(pr, s)

                    PROD = {5: ("v", 8), 6: ("v", 10), 9: ("v", 12),
                            10: ("v", 14)}

                    def attn_block_p(prs, s_lo, s_hi):
                        for s in range(s_lo, s_hi):
                            for pr in prs:
                                score_exp(pr, s)
                            if s in PROD:
                                v_pair(PROD[s][1])
                            if s >= LAG:
                                for pr in prs:
                                    pv_pair(pr, s - LAG)

                    alloc_po(0)
                    alloc_po(1)
                    attn_block_p((0, 1), 0, 8)
                    kt_pair(2, 0)
                    kt_pair(2, 1)
                    attn_block_p((0, 1), 8, 12)
                    kt_pair(3, 0)
                    kt_pair(3, 1)
                    attn_block_p((0, 1), 12, 16)
                    attn_drain((0, 1))
                    # keep the exp stream fed through pass 1's tails
                    attn_block((2, 3), 0, LAG)
                    attn_tail(0)
                    attn_tail(1)
                    alloc_po(2)
                    alloc_po(3)
                    attn_block((2, 3), LAG, 16)
                    attn_drain((2, 3))
                    attn_tail(2)
                    attn_tail(3)

            # ======== output projection + residual ========
            with tc.tile_pool(name="pf", bufs=2, space="PSUM") as pf_ps:
                for qt in range(NQ):
                    ps = pf_ps.tile([128, 512], F32, tag="pf")
                    nc.tensor.matmul(ps[:], ones_sb[0:1, 0:128],
                                     bo_sb[:], start=True, stop=False)
                    for cc in range(4):
                        nc.tensor.matmul(
                            ps[:], at_sb[:, cc, bass.ts(qt, 128)],
                            wo_sb[:, cc, :], start=False, stop=(cc == 3))
                    nc.vector.tensor_add(out=x2_sb[:, qt, :], in0=ps[:],
                                         in1=x_sb[:, 12 + qt, :])

            # ======== LN2 + transpose ========
            with tc.tile_pool(name="ptr2", bufs=2, space="PSUM") as ptr2:
                for qt in range(NQ):
                    ht = tp.tile([128, 512], BF16, tag="ht")
                    layernorm_to(x2_sb[:, qt, :], ht[:])
                    pst = ptr2.tile([128, 4, 128], BF16, tag="tr")
                    for cc in range(4):
                        nc.tensor.transpose(
                            pst[:, cc, :], ht[:, bass.ts(cc, 128)], ident[:])
                    ev = h2t_sb[:, :, bass.ts(qt, 128)]
                    nc.scalar.copy(ev, pst[:])

            # ======== FFN1 + gelu + FFN2, fused per f-tile ========
            with (
                tc.tile_pool(name="pg", bufs=2, space="PSUM") as pg_ps,
                tc.tile_pool(name="pf2", bufs=1, space="PSUM") as pf2_ps,
                tc.tile_pool(name="gp", bufs=3) as gp,
                tc.tile_pool(name="op", bufs=2) as op,
            ):
                pso = [pf2_ps.tile([128, 512], F32, tag=f"o{qt}",
                                   name=f"o{qt}") for qt in range(NQ)]
                for qt in range(NQ):
                    nc.tensor.matmul(
                        pso[qt][:], ones_sb[0:1, 0:128], b2_sb[:],
                        start=True, stop=False, skip_group_check=True)
                for f in range(16):
                    ps = pg_ps.tile([128, 512], F32, tag="pg")
                    for cc in range(4):
                        nc.tensor.matmul(
                            ps[:], w1_sb[:, f, bass.ts(cc, 128)],
                            h2t_sb[:, cc, :],
                            start=(cc == 0), stop=(cc == 3))
                    gt = gp.tile([128, 512], BF16, tag="gt")
                    nc.scalar.activation(
                        out=gt[:], in_=ps[:], func=AF.Gelu,
                        bias=b1_sb[:, f:f + 1])
                    for qt in range(NQ):
                        nc.tensor.matmul(
                            pso[qt][:],
                            gt[:, bass.ts(qt, 128)],
                            w2_sb[:, f, :], start=False, stop=(f == 15),
                            skip_group_check=True)
                for qt in range(NQ):
                    ot = op.tile([128, 512], F32, tag="ot")
                    nc.vector.tensor_add(out=ot[:], in0=pso[qt][:],
                                         in1=x2_sb[:, qt, :])
                    eng = nc.sync if qt % 2 == 0 else nc.scalar
                    eng.dma_start(out_d[bass.ts(qt, 128), :], ot[:])

    nc.compile()
    return nc


def _bf16(a):
    return np.ascontiguousarray(np.asarray(a, np.float32)).astype(
        ml_dtypes.bfloat16)


def _host_prep(x, Wq, Wk, Wv, Wo, bo, W1, b1, W2, b2, g1, be1, g2, be2):
    """Fold LN gains into weights; build per-core rotated inputs/slot biases."""
    x = np.asarray(x, np.float32)
    g1 = np.asarray(g1, np.float32)
    be1 = np.asarray(be1, np.float32)
    g2 = np.asarray(g2, np.float32)
    be2 = np.asarray(be2, np.float32)

    wq_cat = np.transpose(np.asarray(Wq, np.float32), (1, 0, 2)).reshape(C, H * HD)
    wk_cat = np.transpose(np.asarray(Wk, np.float32), (1, 0, 2)).reshape(C, H * HD)
    wv_cat = np.transpose(np.asarray(Wv, np.float32), (1, 0, 2)).reshape(C, H * HD)
    scl = float(HD) ** -0.5
    wq_f = (g1[:, None] * wq_cat) * scl
    wk_f = g1[:, None] * wk_cat
    wv_f = g1[:, None] * wv_cat
    bq = (be1 @ wq_cat) * scl
    bk = be1 @ wk_cat
    bv = be1 @ wv_cat
    with_qkv_bias = bool(np.any(bq) or np.any(bk) or np.any(bv))

    W1 = np.asarray(W1, np.float32)
    w1_f = g2[:, None] * W1
    b1_f = np.asarray(b1, np.float32) + be2 @ W1

    # 0/1 keep-mask for the diagonal block in S^T layout, both halves
    maskc = np.zeros((4, 128, 512), np.float32)
    qidx = np.arange(512)[None, :]
    for dd in range(4):
        pidx = 128 * dd + np.arange(128)[:, None]
        maskc[dd] = (pidx <= qidx).astype(np.float32)
    mask4 = np.broadcast_to(
        maskc.transpose(1, 0, 2)[:, :, None, :], (128, 4, 2, 512))

    common = {
        "identc": _bf16(np.eye(128, dtype=np.float32)),
        "maskc": _bf16(mask4),
        "onesc": _bf16(np.ones((128, 512), np.float32)),
        "wq": _bf16(wq_f.reshape(4, 128, 512).transpose(1, 0, 2)),
        "wk": _bf16(wk_f.reshape(4, 128, 512).transpose(1, 0, 2)),
        "wv": _bf16(wv_f.reshape(4, 128, 512).transpose(1, 0, 2)),
        "wo": _bf16(
            np.asarray(Wo, np.float32).reshape(4, 128, 512).transpose(1, 0, 2)),
        "w1": _bf16(
            np.transpose(
                w1_f.reshape(4, 128, 16, 128).transpose(2, 1, 0, 3)
                .reshape(16, 128, 512), (1, 0, 2))),
        "w2": _bf16(np.transpose(
            np.asarray(W2, np.float32).reshape(16, 128, 512), (1, 0, 2))),
        "bo": _bf16(np.asarray(bo, np.float32).reshape(1, 512)),
        "b1c": np.ascontiguousarray(b1_f.reshape(16, 128).T),
        "b2r": _bf16(np.asarray(b2, np.float32).reshape(1, 512)),
    }
    if with_qkv_bias:
        common["bqkv"] = _bf16(np.stack([bq, bk, bv]).reshape(3, 1, 512))

    in_maps = []
    for c in range(NCORES):
        bb, j = c // 4, c % 4
        o = QB * j
        xb_rot = np.roll(x[bb], 1536 - o, axis=0)
        kbias = np.zeros(16, np.float32)
        for s in range(12):
            m = (s - 12 + 4 * j) % 16
            if m >= 4 * j:       # original key tile at/after the q block
                kbias[s] = NEG
        im = dict(common)
        im["xb"] = np.ascontiguousarray(xb_rot)
        im["kbias"] = np.ascontiguousarray(
            np.broadcast_to(kbias.reshape(1, 16), (128, 16)))
        in_maps.append(im)
    return in_maps, with_qkv_bias


def kernel(**inputs):
    global last_run
    in_maps, with_qkv_bias = _host_prep(**inputs)
    if with_qkv_bias not in _prog_cache:
        _prog_cache[with_qkv_bias] = _build_program(with_qkv_bias)
    nc = _prog_cache[with_qkv_bias]
    res = run_bass_kernel_spmd(nc, in_maps, list(range(NCORES)))
    last_run = res
    out = np.empty((B, T, C), np.float32)
    for c in range(NCORES):
        bb, j = c // 4, c % 4
        out[bb, QB * j:QB * (j + 1), :] = res.results[c]["out"]
    return out
